# revision 39
# baseline (speedup 1.0000x reference)
"""Self-contained Trainium2 Bass kernel for the nn_EnocoderBlock problem.

kernel(**inputs) takes the full (unsharded) inputs of the reference encoder
block (B=2, S=2048, D=1024, H=16, DFF=4096) and returns the full [B, S, D]
fp32 output, running SPMD on 8 NeuronCores.

Sharding: data-parallel over batch x query-token blocks — each of the 8
cores owns one batch element's full K/V context and a 512-token query
slice, so no cross-core collectives are needed (K/V projections are
recomputed by the 4 cores sharing a batch element).

Precision: QKV/attention/O-proj matmuls run in fp8e4m3 with DoubleRow
perf mode (two 128-deep k-tiles per instruction) and fp32 PSUM
accumulation; Q/K/V/ctx/exp tiles are stored fp8; the FFN runs in bf16;
softmax statistics, residuals and LayerNorms are fp32.  LayerNorm uses
one-pass stats (sum via DVE reduce, sum-of-squares via ACT Square
accumulate, var = E[x^2]-mean^2) so mean/var never serialize behind a
centering pass.
"""

import sys
for _p in ("/opt/trn_rl_repo", "/root/.axon_site/_ro/trn_rl_repo"):
    if _p not in sys.path:
        sys.path.append(_p)

import numpy as np

import math
from contextlib import ExitStack

import concourse.mybir as mybir
import concourse.tile as tile
from concourse.bass import ds, ts
from concourse.masks import make_identity

F32 = mybir.dt.float32
BF16 = mybir.dt.bfloat16
FP8 = mybir.dt.float8e4
AX = mybir.AxisListType
ALU = mybir.AluOpType
ACTF = mybir.ActivationFunctionType
DR = mybir.MatmulPerfMode.DoubleRow

P = 128
EPS = 1e-6


def build(nc, S=2048, D=1024, H=16, DK=64, DFF=4096, TQ=512):
    assert DK == 64 and D % P == 0 and S % P == 0 and DFF % P == 0
    NJ = D // P            # feature tiles of 128
    NT = S // P            # token tiles of 128 (full seq)
    NTQ = TQ // P          # query token tiles of 128
    TN = 512               # moving-dim tile for token axis
    NTN = S // TN
    QN = 512               # moving-dim tile for query axis
    NQN = TQ // QN
    NF = DFF // P          # dff tiles of 128
    HPJ = P // DK          # heads per 128-feature tile (=2)
    HG = 2                 # attention head-group size

    # ---------------- DRAM I/O ----------------
    def din(name, shape, dt=FP8):
        return nc.dram_tensor(name, shape, dt, kind="ExternalInput").ap()

    xT, xTq, xq = din("xT", [D, S]), din("xTq", [D, TQ]), din("xq", [TQ, D], F32)
    wqT, wkT = din("wqT", [D, D]), din("wkT", [D, D])
    wvT, woT = din("wvT", [D, D]), din("woT", [D, D])
    w1T, w2T = din("w1T", [D, DFF], BF16), din("w2T", [DFF, D], BF16)
    cpm = din("cpm", [P, 2 * (D // P) + DFF // P], F32)   # bq|bk|b1, partition-major
    crow = din("crow", [1, 2 * D + 2], F32)               # bv|b2|alpha|gamma
    out = nc.dram_tensor("out", [TQ, D], F32, kind="ExternalOutput").ap()

    # partition-major views (p = inner index of leading dim)
    xT_v = xT.rearrange("(o p) t -> p o t", p=P)          # [128, NJ, S]
    xTq_v = xTq.rearrange("(o p) t -> p o t", p=P)
    xq_v = xq.rearrange("(o p) d -> p o d", p=P)          # [128, NTQ, D]
    out_v = out.rearrange("(o p) d -> p o d", p=P)
    wqT_v = wqT.rearrange("(o p) j -> p o j", p=P)        # [128, NJ, D]
    wkT_v = wkT.rearrange("(o p) j -> p o j", p=P)
    wvT_v = wvT.rearrange("(o p) j -> p o j", p=P)
    woT_v = woT.rearrange("(o p) j -> p o j", p=P)
    w1T_v = w1T.rearrange("(o p) f -> p o f", p=P)        # [128, NJ, DFF]
    w2T_v = w2T.rearrange("(o p) j -> p o j", p=P)        # [128, NF, D]

    with tile.TileContext(nc) as tc, ExitStack() as octx:
        small = octx.enter_context(tc.tile_pool(name="small", bufs=1))

        # ------------- left-stack pools (LIFO per side) -------------
        # small < ef(out1) < ev(xq + LN scratch) < ctx < kq < v < xt
        ef_cm = tc.tile_pool(name="efpool", bufs=1)
        ef = ef_cm.__enter__()
        out1_sb = ef.tile([P, NTQ, D], F32, tag="out1")
        out1T_sb = ef.tile([P, NJ, TQ], BF16, tag="out1T")

        ev_cm = tc.tile_pool(name="evpool", bufs=2)
        ev = ev_cm.__enter__()
        xq_sb = ev.tile([P, NTQ, D], F32, tag="xq", bufs=1)

        ctx_cm = tc.tile_pool(name="ctxpool", bufs=1)
        ctxp = ctx_cm.__enter__()
        ctx_sb = ctxp.tile([P, NJ, TQ], FP8, tag="ctx")
        ON = 512
        NON = D // ON

        kq_cm = tc.tile_pool(name="kq", bufs=1)
        kq = kq_cm.__enter__()
        K_sb = kq.tile([P, NJ, S], FP8, tag="K")
        Q_sb = kq.tile([P, NJ, TQ], FP8, tag="Q")

        v_cm = tc.tile_pool(name="vpool", bufs=1)
        vp = v_cm.__enter__()
        V_sb = vp.tile([P, NT, H, DK + 1], FP8, tag="V")

        expool_cm = tc.tile_pool(name="expool", bufs=3)
        expool = expool_cm.__enter__()

        xt_cm = tc.tile_pool(name="xtpool", bufs=1)
        xtp = xt_cm.__enter__()
        wq_all = xtp.tile([P, NJ, D], FP8, tag="wq_all")
        wk_all = xtp.tile([P, NJ, D], FP8, tag="wk_all")
        wv_all = xtp.tile([P, NJ, D], FP8, tag="wv_all")
        xTq_sb = xtp.tile([P, NJ, TQ], FP8, tag="xTq")
        xt_all = xtp.tile([P, NJ, S], FP8, tag="xt_all")

        # ---- DMA prefetch order (sync queue serializes in issue order) ----
        XCH = S // 4

        def xchunk(xc):
            nc.sync.dma_start(xt_all[:, :, ds(xc * XCH, XCH)],
                              xT_v[:, :, ds(xc * XCH, XCH)])

        nc.sync.dma_start(xTq_sb[:], xTq_v)
        nc.sync.dma_start(wq_all[:, :, 0:D // 2], wqT_v[:, :, 0:D // 2])
        cpm_sb = small.tile([P, 2 * NJ + NF], F32, tag="cpm")
        nc.sync.dma_start(cpm_sb[:], cpm)
        bq_sb, bk_sb = cpm_sb[:, 0:NJ], cpm_sb[:, NJ:2 * NJ]
        b1_sb = cpm_sb[:, 2 * NJ:2 * NJ + NF]
        crow_sb = small.tile([1, 2 * D + 2], F32, tag="crow")
        nc.sync.dma_start(crow_sb[:], crow)
        nc.sync.dma_start(wq_all[:, :, ds(D // 2, D // 2)],
                          wqT_v[:, :, ds(D // 2, D // 2)])
        nc.sync.dma_start(wk_all[:], wkT_v)
        for xc in range(4):
            xchunk(xc)
        nc.sync.dma_start(wv_all[:], wvT_v)
        wo_sb = small.tile([P, NJ, D], FP8, tag="wo")
        nc.sync.dma_start(wo_sb[:], woT_v)

        # ---------------- constants / broadcast rows ----------------
        ident = small.tile([P, P], F32, tag="ident")
        make_identity(nc, ident)
        ident_bf = small.tile([P, P], BF16, tag="ident_bf")
        nc.vector.tensor_copy(ident_bf[:], ident[:])
        ones_bf = small.tile([1, P], BF16, tag="ones_bf")
        nc.vector.memset(ones_bf[:], 1.0)
        b2row_bf = small.tile([1, D], BF16, tag="b2row_bf")
        nc.vector.tensor_copy(b2row_bf[:], crow_sb[:, ds(D, D)])

        bv_bc = small.tile([P, D], F32, tag="bv_bc")
        nc.gpsimd.partition_broadcast(bv_bc[:], crow_sb[:, 0:D])
        ag_bc = small.tile([P, 2], F32, tag="ag_bc")
        nc.gpsimd.partition_broadcast(ag_bc[:], crow_sb[:, ds(2 * D, 2)])
        alpha_bc = ag_bc[:, 0:1]
        gamma_bc = ag_bc[:, 1:2]

        eps_bc = small.tile([P, 1], F32, tag="eps_bc")
        nc.vector.memset(eps_bc[:], EPS)

        # ------------- phase A helpers (Q, K, V; fp8 DoubleRow) -------
        # Attention score/c2 psum pools open first so score emission can be
        # interleaved into the projection stream (PSUM stack: d,c2,a then
        # a closes and o,ef open -> never more than 8 banks).
        psum_d_cm = tc.tile_pool(name="psum_d", bufs=2, space="PSUM")
        psum_d = psum_d_cm.__enter__()
        psum_c2_cm = tc.tile_pool(name="psum_c2", bufs=1, space="PSUM")
        psum_c2 = psum_c2_cm.__enter__()
        psum_a_cm = tc.tile_pool(name="psum_a", bufs=3, space="PSUM")
        psum_a = psum_a_cm.__enter__()

        VN = 512
        NVN = D // VN
        HPV = VN // DK

        def qproj():
            for jt in range(NJ):
                ps = psum_a.tile([P, QN], F32, tag="ps")
                for kp in range(NJ // 2):
                    nc.tensor.matmul(
                        ps[:], wq_all[:, 2 * kp:2 * kp + 2, ts(jt, P)],
                        xTq_sb[:, 2 * kp:2 * kp + 2, :],
                        start=(kp == 0), stop=(kp == NJ // 2 - 1),
                        perf_mode=DR)
                nc.vector.tensor_scalar_add(
                    Q_sb[:, jt, :], ps[:], bq_sb[:, jt:jt + 1])

        def kproj(jt, nts=None):
            for nt in (range(NTN) if nts is None else nts):
                ps = psum_a.tile([P, TN], F32, tag="ps")
                for kp in range(NJ // 2):
                    nc.tensor.matmul(
                        ps[:], wk_all[:, 2 * kp:2 * kp + 2, ts(jt, P)],
                        xt_all[:, 2 * kp:2 * kp + 2, ds(nt * TN, TN)],
                        start=(kp == 0), stop=(kp == NJ // 2 - 1),
                        perf_mode=DR)
                nc.vector.tensor_scalar_add(
                    K_sb[:, jt, ds(nt * TN, TN)], ps[:], bk_sb[:, jt:jt + 1])

        def vproj(nv, tts):
            for tt in tts:
                ps = psum_a.tile([P, VN], F32, tag="ps")
                for kp in range(NJ // 2):
                    nc.tensor.matmul(
                        ps[:], xt_all[:, 2 * kp:2 * kp + 2, ts(tt, P)],
                        wv_all[:, 2 * kp:2 * kp + 2, ds(nv * VN, VN)],
                        start=(kp == 0), stop=(kp == NJ // 2 - 1),
                        perf_mode=DR)
                nc.vector.tensor_tensor(
                    V_sb[:, tt, ds(nv * HPV, HPV), 0:DK],
                    ps[:].rearrange("p (h d) -> p h d", d=DK),
                    bv_bc[:, ds(nv * VN, VN)].rearrange(
                        "p (h d) -> p h d", d=DK),
                    ALU.add)

        nc.sync.dma_start(xq_sb[:], xq_v)

        # ------- phases D/E/F: query-half pipelined attention + FFN -------
        # Attention runs per query half (HG=1, score chunks of 4 token
        # tiles in a 2-bank PSUM, exp -> fp8, attn.V as DoubleRow pairs).
        # While half 1's exp-bound attention occupies ACT, the PE stream is
        # filled with half 0's FFN1 strips.  O-projection accumulates all 4
        # fp8-DR k-pairs in PSUM and flushes straight into the residual.
        # FFN2 runs at the tail against streamed w2 with all 8 PSUM banks
        # as per-(tt,no) accumulators seeded with out1.
        QH2 = TQ // 2          # 256 queries per half
        CK = 4                 # score chunk: 4 token tiles per psum_d tile
        NCK = NT // CK

        exmap = {}

        def att_scores(qh, h, cks=None):
            hp = (h % HPJ) * DK
            hj = h // HPJ
            qsl = ds(qh * QH2, QH2)
            for ck in (range(NCK) if cks is None else cks):
                ps = psum_d.tile([P, CK, QH2], F32, tag="ps2")
                for i in range(CK):
                    nc.tensor.matmul(
                        ps[:, i], K_sb[ds(hp, DK), hj, ts(ck * CK + i, P)],
                        Q_sb[ds(hp, DK), hj, qsl],
                        start=True, stop=True)
                ex = expool.tile([P, CK, QH2], FP8, tag="ex",
                                 bufs=8, name=f"ex_{qh}_{h}_{ck}")
                nc.scalar.activation(ex[:], ps[:], ACTF.Exp,
                                     scale=1.0 / math.sqrt(DK))
                exmap[(qh, h, ck)] = ex

        def att_av(qh, h):
            hp = (h % HPJ) * DK
            hj = h // HPJ
            qsl = ds(qh * QH2, QH2)
            c2 = psum_c2.tile([P, QH2], F32, tag="c2", name=f"c2_{qh}_{h}")
            for ck in range(NCK):
                ex = exmap.pop((qh, h, ck))
                for i in range(CK // 2):
                    mt0 = ck * CK + 2 * i
                    nc.tensor.matmul(
                        c2[0:DK + 1, :],
                        V_sb[:, mt0:mt0 + 2, h, :], ex[:, 2 * i:2 * i + 2],
                        start=(mt0 == 0), stop=(mt0 == NT - 2),
                        perf_mode=DR)
            recip = expool.tile([1, QH2], F32, tag="recip", bufs=2)
            nc.vector.reciprocal(recip[:], c2[DK:DK + 1, :])
            recip_bc = expool.tile([DK, QH2], F32, tag="recip_bc", bufs=2)
            nc.gpsimd.partition_broadcast(recip_bc[:], recip[:])
            nc.vector.tensor_tensor(ctx_sb[ds(hp, DK), hj, qsl],
                                    c2[0:DK, :], recip_bc[:], ALU.mult)

        def attention(qh, h):
            hp = (h % HPJ) * DK
            hj = h // HPJ
            qsl = ds(qh * QH2, QH2)
            c2 = psum_c2.tile([P, QH2], F32, tag="c2", name=f"c2_{qh}_{h}")
            exs = []
            for ck in range(NCK + 1):
                if ck < NCK:
                    ps = psum_d.tile([P, CK, QH2], F32, tag="ps2")
                    for i in range(CK):
                        nc.tensor.matmul(
                            ps[:, i], K_sb[ds(hp, DK), hj, ts(ck * CK + i, P)],
                            Q_sb[ds(hp, DK), hj, qsl],
                            start=True, stop=True)
                    ex = expool.tile([P, CK, QH2], FP8, tag="ex",
                                     bufs=8, name=f"ex_{qh}_{h}_{ck}")
                    nc.scalar.activation(ex[:], ps[:], ACTF.Exp,
                                         scale=1.0 / math.sqrt(DK))
                    exs.append(ex)
                if ck >= 1:
                    ex = exs[ck - 1]
                    for i in range(CK // 2):
                        mt0 = (ck - 1) * CK + 2 * i
                        nc.tensor.matmul(
                            c2[0:DK + 1, :],
                            V_sb[:, mt0:mt0 + 2, h, :], ex[:, 2 * i:2 * i + 2],
                            start=(mt0 == 0), stop=(mt0 == NT - 2),
                            perf_mode=DR)
            recip = expool.tile([1, QH2], F32, tag="recip", bufs=2)
            nc.vector.reciprocal(recip[:], c2[DK:DK + 1, :])
            recip_bc = expool.tile([DK, QH2], F32, tag="recip_bc", bufs=2)
            nc.gpsimd.partition_broadcast(recip_bc[:], recip[:])
            nc.vector.tensor_tensor(ctx_sb[ds(hp, DK), hj, qsl],
                                    c2[0:DK, :], recip_bc[:], ALU.mult)

        def oproj_ln(qh):
            # O-projection + residual + LN1 + transpose for one query half
            for t2 in range(2):
                tt = qh * 2 + t2
                for no in range(NON):
                    pso = psum_o.tile([P, ON], F32, tag="pso",
                                      name=f"pso_{qh}_{t2}_{no}")
                    for kp in range(NJ // 2):
                        nc.tensor.matmul(
                            pso[:], ctx_sb[:, 2 * kp:2 * kp + 2, ts(tt, P)],
                            wo_sb[:, 2 * kp:2 * kp + 2, ds(no * ON, ON)],
                            start=(kp == 0), stop=(kp == NJ // 2 - 1),
                            perf_mode=DR)
                    sl = ds(no * ON, ON)
                    nc.vector.tensor_tensor(out1_sb[:, tt, sl], pso[:],
                                            xq_sb[:, tt, sl], ALU.add)
                o1 = out1_sb[:, tt, :]
                _layer_norm(nc, ev, o1, o1, D, alpha_bc, gamma_bc, eps_bc)
                for jt in range(NJ):
                    pst = psum_ef.tile([P, QH2], F32, tag="ps1")
                    nc.tensor.transpose(
                        pst[:, 0:P], out1_sb[:, tt, ts(jt, P)], ident[:])
                    nc.vector.tensor_copy(out1T_sb[:, jt, ts(tt, P)],
                                          pst[:, 0:P])

        def ffn1_strip(qh, mp):
            qsl = ds(qh * QH2, QH2)
            w1_col = fstream.tile([P, NJ, 2 * P], BF16, tag="w1_col",
                                  name=f"w1c_{qh}_{mp}")
            nc.sync.dma_start(w1_col[:], w1T_v[:, :, ds(mp * 2 * P, 2 * P)])
            for mi in range(2):
                mt = mp * 2 + mi
                ps = psum_ef.tile([P, QH2], F32, tag="ps1")
                for kt in range(NJ):
                    nc.tensor.matmul(
                        ps[:], w1_col[:, kt, ts(mi, P)],
                        out1T_sb[:, kt, qsl],
                        start=(kt == 0), stop=(kt == NJ - 1))
                nc.vector.tensor_scalar(hid_sb[:, mt, qsl], ps[:],
                                        b1_sb[:, mt:mt + 1], 0.0,
                                        ALU.add, op1=ALU.max)

        # ---- phase A interleaved with half 0's attention ----
        # scores for head 0 are emitted right after K's first feature tile
        # so exp (the critical ACT stream) starts as early as possible; the
        # remaining K/V projection work fills the PE while ACT exps.
        nc.vector.memset(V_sb[:, :, :, DK:DK + 1], 1.0)
        qproj()
        # K feature tile 0 per token-chunk, each feeding head 0's matching
        # score chunk -> the exp stream (the ACT critical path) starts as
        # soon as the first xT chunk lands
        for ck in range(NCK):
            kproj(0, [ck])
            att_scores(0, 0, [ck])
        kproj(1)
        att_scores(0, 1)
        vproj(0, range(NT))              # heads 0-7's V (wv DMA-gated)
        att_av(0, 0)
        att_scores(0, 2)
        att_av(0, 1)
        for jt in range(2, NJ):
            kproj(jt)
            if jt < NJ - 1:
                att_scores(0, jt + 1)
                att_av(0, jt)
        att_av(0, NJ - 1)
        vproj(1, range(NT))              # heads 8-15's V
        psum_a_cm.__exit__(None, None, None)
        psum_o_cm = tc.tile_pool(name="psum_o", bufs=1, space="PSUM")
        psum_o = psum_o_cm.__enter__()
        psum_ef_cm = tc.tile_pool(name="psum_ef", bufs=2, space="PSUM")
        psum_ef = psum_ef_cm.__enter__()
        xt_cm.__exit__(None, None, None)    # release xT + proj weights
        fpool_cm = tc.tile_pool(name="fpool", bufs=1, side="right")
        fpool = fpool_cm.__enter__()
        hid_sb = fpool.tile([P, NF, TQ], BF16, tag="hid")
        fstream_cm = tc.tile_pool(name="fstream", bufs=4, side="right")
        fstream = fstream_cm.__enter__()
        w2_blk0 = fstream.tile([P, 4, D], BF16, tag="w2_blk0", bufs=1)
        nc.gpsimd.dma_start(w2_blk0[:], w2T_v[:, 0:4, :])
        for h in range(H // 2, H):
            attention(0, h)
        oproj_ln(0)
        # ---- half 1: attention interleaved with half 0's FFN1 ----
        for h in range(H):
            attention(1, h)
            ffn1_strip(0, h)
        oproj_ln(1)
        for mp in range(NF // 2):
            ffn1_strip(1, mp)
        psum_ef_cm.__exit__(None, None, None)
        psum_o_cm.__exit__(None, None, None)
        psum_c2_cm.__exit__(None, None, None)
        psum_d_cm.__exit__(None, None, None)
        expool_cm.__exit__(None, None, None)

        v_cm.__exit__(None, None, None)     # release V
        kq_cm.__exit__(None, None, None)    # release K, Q

        # ---------------- FFN2 tail: streamed w2, 8 psum accumulators ------
        KTG = 4
        NKTG = NF // KTG
        f2s_cm = tc.tile_pool(name="f2stream", bufs=2, side="right")
        f2s = f2s_cm.__enter__()
        psum_f2_cm = tc.tile_pool(name="psum_f2", bufs=8, space="PSUM")
        psum_f2 = psum_f2_cm.__enter__()
        psf = {}
        for tt in range(NTQ):
            for no in range(NON):
                ps = psum_f2.tile([P, ON], F32, tag="psf",
                                  name=f"psf_{tt}_{no}")
                psf[(tt, no)] = ps
                for jj in range(ON // P):
                    jt = no * (ON // P) + jj
                    nc.tensor.matmul(
                        ps[:, ds(jj * P, P)], out1T_sb[:, jt, ts(tt, P)],
                        ident_bf[:], start=(jj == 0), stop=False,
                        skip_group_check=True)
                nc.tensor.matmul(
                    ps[:], ones_bf[:], b2row_bf[:, ds(no * ON, ON)],
                    start=False, stop=False, skip_group_check=True)
        for ktg in range(NKTG - 1):
            if ktg == 0:
                w2_blk = w2_blk0
            else:
                w2_blk = f2s.tile([P, KTG, D], BF16, tag="w2_blk")
                nc.gpsimd.dma_start(w2_blk[:], w2T_v[:, ds(ktg * KTG, KTG), :])
            for tt in range(NTQ):
                for no in range(NON):
                    ps = psf[(tt, no)]
                    for kk in range(KTG):
                        kt = ktg * KTG + kk
                        nc.tensor.matmul(
                            ps[:], hid_sb[:, kt, ts(tt, P)],
                            w2_blk[:, kk, ds(no * ON, ON)],
                            start=False,
                            stop=False,
                            skip_group_check=True)
        # last block: per-tile finish so each LN2 chain overlaps the next
        # tile's matmuls (short drain)
        w2_blk = f2s.tile([P, KTG, D], BF16, tag="w2_blk")
        nc.gpsimd.dma_start(w2_blk[:], w2T_v[:, ds(NF - KTG, KTG), :])
        for tt in range(NTQ):
            res2 = ev.tile([P, D], F32, tag="res2")
            st = ev.tile([P, 16], F32, tag="f2stat")
            sqg = ev.tile([P, ON], BF16, tag="f2sq", bufs=1)
            for no in range(NON):
                ps = psf[(tt, no)]
                for kk in range(KTG):
                    kt = NF - KTG + kk
                    nc.tensor.matmul(
                        ps[:], hid_sb[:, kt, ts(tt, P)],
                        w2_blk[:, kk, ds(no * ON, ON)],
                        start=False, stop=(kt == NF - 1),
                        skip_group_check=True)
                # stats straight off the psum accumulator (b2 was seeded)
                nc.vector.reduce_sum(st[:, no:no + 1], ps[:], axis=AX.X)
                nc.scalar.activation(sqg[:], ps[:], ACTF.Square,
                                     accum_out=st[:, 2 + no:3 + no])
            nc.vector.tensor_tensor(st[:, 4:5], st[:, 0:1], st[:, 1:2],
                                    ALU.add)                       # sum
            nc.vector.tensor_tensor(st[:, 5:6], st[:, 2:3], st[:, 3:4],
                                    ALU.add)                       # sumsq
            nc.vector.tensor_scalar(st[:, 6:7], st[:, 4:5], st[:, 4:5],
                                    1.0 / (D * D), ALU.mult, op1=ALU.mult)
            nc.vector.tensor_scalar(st[:, 7:8], st[:, 5:6], 1.0 / D, EPS,
                                    ALU.mult, op1=ALU.add)
            nc.vector.tensor_tensor(st[:, 8:9], st[:, 7:8], st[:, 6:7],
                                    ALU.subtract)                  # var+eps
            nc.scalar.activation(st[:, 9:10], st[:, 8:9], ACTF.Sqrt)
            nc.vector.reciprocal(st[:, 10:11], st[:, 9:10])
            nc.vector.tensor_scalar(st[:, 11:12], st[:, 10:11], alpha_bc,
                                    None, ALU.mult)                # s
            nc.vector.tensor_scalar(st[:, 12:13], st[:, 11:12], st[:, 4:5],
                                    1.0 / D, ALU.mult, op1=ALU.mult)
            nc.vector.tensor_scalar(st[:, 13:14], st[:, 12:13], -1.0,
                                    gamma_bc, ALU.mult, op1=ALU.add)  # t
            for no in range(NON):
                sl = ds(no * ON, ON)
                nc.vector.tensor_scalar(res2[:, sl], psf[(tt, no)][:],
                                        st[:, 11:12], st[:, 13:14],
                                        ALU.mult, op1=ALU.add)
                nc.sync.dma_start(out_v[:, tt, sl], res2[:, sl])
        psum_f2_cm.__exit__(None, None, None)
        f2s_cm.__exit__(None, None, None)
        fstream_cm.__exit__(None, None, None)
        fpool_cm.__exit__(None, None, None)
        ctx_cm.__exit__(None, None, None)

        ev_cm.__exit__(None, None, None)
        ef_cm.__exit__(None, None, None)

    return nc


def _layer_norm(nc, pool, out_ap, x_ap, D, alpha_bc, gamma_bc, eps_bc,
                use_sqrt=False, final_pool=False):
    """out = alpha * (x - mean) / sqrt(var + EPS) + gamma, stats over free dim.

    One-pass stats: sum on DVE and sum-of-squares on ACT run in parallel;
    var = sumsq/D - mean^2; out = x*(alpha*rstd) + (gamma - mean*alpha*rstd)
    is a single ACT pass over x with per-partition scale/bias.
    """
    stat = pool.tile([P, 8], F32, tag="ln_stat")
    sq = pool.tile([P, D], BF16, tag="ln_sq", bufs=1)
    nc.vector.reduce_sum(stat[:, 0:1], x_ap, axis=AX.X)
    nc.scalar.activation(sq[:], x_ap, ACTF.Square, accum_out=stat[:, 2:3])
    # m2 = (sum/D)^2 ; ve = sumsq/D + eps ; var' = ve - m2
    nc.vector.tensor_scalar(stat[:, 3:4], stat[:, 0:1], stat[:, 0:1],
                            1.0 / (D * D), ALU.mult, op1=ALU.mult)
    nc.vector.tensor_scalar(stat[:, 4:5], stat[:, 2:3], 1.0 / D, EPS,
                            ALU.mult, op1=ALU.add)
    nc.vector.tensor_tensor(stat[:, 5:6], stat[:, 4:5], stat[:, 3:4],
                            ALU.subtract)
    rstd = pool.tile([P, 8], F32, tag="ln_rstd")
    v = stat[:, 5:6]
    z = rstd[:, 1:2]
    if use_sqrt:
        # tail path: exp is over, one table switch to sqrt is cheap and
        # much shorter than the Newton chain
        nc.scalar.activation(rstd[:, 0:1], v, ACTF.Sqrt)
        nc.vector.reciprocal(z, rstd[:, 0:1])
    else:
        # mid-kernel path: 1/sqrt(var) via reciprocal seed + 3 Newton
        # iterations, entirely off the ACT table path (no table swaps
        # around the softmax exp stream)
        nc.vector.reciprocal(rstd[:, 0:1], v)
        nc.vector.tensor_scalar(z, rstd[:, 0:1], 0.5, 0.5, ALU.mult,
                                op1=ALU.add)
        for it in range(3):
            u = rstd[:, 2 + it:3 + it]
            nc.vector.tensor_tensor(u, z, z, ALU.mult)        # z^2
            nc.vector.tensor_scalar(u, u, v, None, ALU.mult)  # v*z^2
            nc.vector.tensor_scalar(u, u, -0.5, 1.5, ALU.mult, op1=ALU.add)
            nc.vector.tensor_scalar(z, z, u, None, ALU.mult)
    s = rstd[:, 5:6]
    nc.vector.tensor_scalar(s, z, alpha_bc, None, ALU.mult)
    # t = gamma - (sum/D)*s
    nc.vector.tensor_scalar(rstd[:, 6:7], s, stat[:, 0:1],
                            1.0 / D, ALU.mult, op1=ALU.mult)
    nc.vector.tensor_scalar(rstd[:, 7:8], rstd[:, 6:7], -1.0, gamma_bc,
                            ALU.mult, op1=ALU.add)
    nc.vector.tensor_scalar(out_ap, x_ap, s, rstd[:, 7:8],
                            ALU.mult, op1=ALU.add)

_B, _S, _D, _H, _DK, _DFF = 2, 2048, 1024, 16, 64, 4096
_NCORES = 8
_TQ = (_B * _S) // _NCORES    # 512 query tokens per core

_cache = {}


def _get_program():
    if "nc" not in _cache:
        from concourse import bacc
        nc = bacc.Bacc("TRN2", target_bir_lowering=False, debug=False,
                       num_devices=_NCORES)
        build(nc, S=_S, D=_D, H=_H, DK=_DK, DFF=_DFF, TQ=_TQ)
        nc.compile()
        _cache["nc"] = nc
    return _cache["nc"]


def _core_inputs(inp):
    """Host-side prep: per-core input dicts (transposes + dtype casts only)."""
    import ml_dtypes
    bf = ml_dtypes.bfloat16
    f8 = ml_dtypes.float8_e4m3

    def t_cast(a, dt):
        return np.ascontiguousarray(np.asarray(a).T).astype(dt)

    bo = np.asarray(inp["bo"])
    f32 = np.float32
    pm = lambda a: np.asarray(a, f32).reshape(-1, 128).T   # partition-major
    cpm = np.ascontiguousarray(np.concatenate(
        [pm(inp["bq"]), pm(inp["bk"]), pm(inp["b1"])], axis=1))
    crow = np.concatenate(
        [np.asarray(inp["bv"], f32), np.asarray(inp["b2"], f32),
         np.asarray(inp["alpha"], f32),
         np.asarray(inp["gamma"], f32)])[None, :]
    w = {
        "wqT": t_cast(inp["wq"], f8), "wkT": t_cast(inp["wk"], f8),
        "wvT": t_cast(inp["wv"], f8), "woT": t_cast(inp["wo"], f8),
        "w1T": t_cast(inp["w1"], bf), "w2T": t_cast(inp["w2"], bf),
        "cpm": cpm, "crow": np.ascontiguousarray(crow),
    }
    x = np.asarray(inp["x"])
    per_batch = _NCORES // _B
    maps = []
    for c in range(_NCORES):
        b, q0 = c // per_batch, (c % per_batch) * _TQ
        xb = x[b]
        m = dict(w)
        m["xT"] = np.ascontiguousarray(xb.T).astype(f8)
        m["xTq"] = np.ascontiguousarray(xb[q0:q0 + _TQ].T).astype(f8)
        # fold the O-projection bias into the residual stream input
        m["xq"] = np.ascontiguousarray(xb[q0:q0 + _TQ]) + bo[None, :]
        maps.append(m)
    return maps


def kernel(**inputs) -> np.ndarray:
    from concourse.bass_utils import run_bass_kernel_spmd
    nc = _get_program()
    in_maps = _core_inputs(inputs)
    res = run_bass_kernel_spmd(nc, in_maps, core_ids=list(range(_NCORES)))
    out = np.empty((_B, _S, _D), dtype=np.float32)
    per_batch = _NCORES // _B
    for c, rm in enumerate(res.results):
        b, q0 = c // per_batch, (c % per_batch) * _TQ
        out[b, q0:q0 + _TQ] = rm["out"]
    return out


# revision 40
# speedup vs baseline: 1.0105x; 1.0105x over previous
"""Self-contained Trainium2 Bass kernel for the nn_EnocoderBlock problem.

kernel(**inputs) takes the full (unsharded) inputs of the reference encoder
block (B=2, S=2048, D=1024, H=16, DFF=4096) and returns the full [B, S, D]
fp32 output, running SPMD on 8 NeuronCores.

Sharding: data-parallel over batch x query-token blocks — each of the 8
cores owns one batch element's full K/V context and a 512-token query
slice, so no cross-core collectives are needed (K/V projections are
recomputed by the 4 cores sharing a batch element).

Precision: QKV/attention/O-proj matmuls run in fp8e4m3 with DoubleRow
perf mode (two 128-deep k-tiles per instruction) and fp32 PSUM
accumulation; Q/K/V/ctx/exp tiles are stored fp8; the FFN runs in bf16;
softmax statistics, residuals and LayerNorms are fp32.  LayerNorm uses
one-pass stats (sum via DVE reduce, sum-of-squares via ACT Square
accumulate, var = E[x^2]-mean^2) so mean/var never serialize behind a
centering pass.
"""

import sys
for _p in ("/opt/trn_rl_repo", "/root/.axon_site/_ro/trn_rl_repo"):
    if _p not in sys.path:
        sys.path.append(_p)

import numpy as np

import math
from contextlib import ExitStack

import concourse.mybir as mybir
import concourse.tile as tile
from concourse.bass import ds, ts
from concourse.masks import make_identity

F32 = mybir.dt.float32
BF16 = mybir.dt.bfloat16
FP8 = mybir.dt.float8e4
AX = mybir.AxisListType
ALU = mybir.AluOpType
ACTF = mybir.ActivationFunctionType
DR = mybir.MatmulPerfMode.DoubleRow

P = 128
EPS = 1e-6


def build(nc, S=2048, D=1024, H=16, DK=64, DFF=4096, TQ=512):
    assert DK == 64 and D % P == 0 and S % P == 0 and DFF % P == 0
    NJ = D // P            # feature tiles of 128
    NT = S // P            # token tiles of 128 (full seq)
    NTQ = TQ // P          # query token tiles of 128
    TN = 512               # moving-dim tile for token axis
    NTN = S // TN
    QN = 512               # moving-dim tile for query axis
    NQN = TQ // QN
    NF = DFF // P          # dff tiles of 128
    HPJ = P // DK          # heads per 128-feature tile (=2)
    HG = 2                 # attention head-group size

    # ---------------- DRAM I/O ----------------
    def din(name, shape, dt=FP8):
        return nc.dram_tensor(name, shape, dt, kind="ExternalInput").ap()

    xT, xTq, xq = din("xT", [D, S]), din("xTq", [D, TQ]), din("xq", [TQ, D], F32)
    wqT, wkT = din("wqT", [D, D]), din("wkT", [D, D])
    wvT, woT = din("wvT", [D, D]), din("woT", [D, D])
    w1T, w2T = din("w1T", [D, DFF], BF16), din("w2T", [DFF, D], BF16)
    cpm = din("cpm", [P, 2 * (D // P) + DFF // P], F32)   # bq|bk|b1, partition-major
    crow = din("crow", [1, 2 * D + 2], F32)               # bv|b2|alpha|gamma
    out = nc.dram_tensor("out", [TQ, D], F32, kind="ExternalOutput").ap()

    # partition-major views (p = inner index of leading dim)
    xT_v = xT.rearrange("(o p) t -> p o t", p=P)          # [128, NJ, S]
    xTq_v = xTq.rearrange("(o p) t -> p o t", p=P)
    xq_v = xq.rearrange("(o p) d -> p o d", p=P)          # [128, NTQ, D]
    out_v = out.rearrange("(o p) d -> p o d", p=P)
    wqT_v = wqT.rearrange("(o p) j -> p o j", p=P)        # [128, NJ, D]
    wkT_v = wkT.rearrange("(o p) j -> p o j", p=P)
    wvT_v = wvT.rearrange("(o p) j -> p o j", p=P)
    woT_v = woT.rearrange("(o p) j -> p o j", p=P)
    w1T_v = w1T.rearrange("(o p) f -> p o f", p=P)        # [128, NJ, DFF]
    w2T_v = w2T.rearrange("(o p) j -> p o j", p=P)        # [128, NF, D]

    with tile.TileContext(nc) as tc, ExitStack() as octx:
        small = octx.enter_context(tc.tile_pool(name="small", bufs=1))

        # ------------- left-stack pools (LIFO per side) -------------
        # small < ef(out1) < ev(xq + LN scratch) < ctx < kq < v < xt
        ef_cm = tc.tile_pool(name="efpool", bufs=1)
        ef = ef_cm.__enter__()
        out1_sb = ef.tile([P, NTQ, D], F32, tag="out1")
        out1T_sb = ef.tile([P, NJ, TQ], BF16, tag="out1T")

        ev_cm = tc.tile_pool(name="evpool", bufs=2)
        ev = ev_cm.__enter__()
        xq_sb = ev.tile([P, NTQ, D], F32, tag="xq", bufs=1)

        ctx_cm = tc.tile_pool(name="ctxpool", bufs=1)
        ctxp = ctx_cm.__enter__()
        ctx_sb = ctxp.tile([P, NJ, TQ], FP8, tag="ctx")
        ON = 512
        NON = D // ON

        kq_cm = tc.tile_pool(name="kq", bufs=1)
        kq = kq_cm.__enter__()
        K_sb = kq.tile([P, NJ, S], FP8, tag="K")
        Q_sb = kq.tile([P, NJ, TQ], FP8, tag="Q")

        v_cm = tc.tile_pool(name="vpool", bufs=1)
        vp = v_cm.__enter__()
        V_sb = vp.tile([P, NT, H, DK + 1], FP8, tag="V")

        expool_cm = tc.tile_pool(name="expool", bufs=3)
        expool = expool_cm.__enter__()

        xt_cm = tc.tile_pool(name="xtpool", bufs=1)
        xtp = xt_cm.__enter__()
        wq_all = xtp.tile([P, NJ, D], FP8, tag="wq_all")
        wk_all = xtp.tile([P, NJ, D], FP8, tag="wk_all")
        wv_all = xtp.tile([P, NJ, D], FP8, tag="wv_all")
        xTq_sb = xtp.tile([P, NJ, TQ], FP8, tag="xTq")
        xt_all = xtp.tile([P, NJ, S], FP8, tag="xt_all")

        # ---- DMA prefetch order (sync queue serializes in issue order) ----
        XCH = S // 4

        def xchunk(xc):
            nc.sync.dma_start(xt_all[:, :, ds(xc * XCH, XCH)],
                              xT_v[:, :, ds(xc * XCH, XCH)])

        nc.sync.dma_start(xTq_sb[:], xTq_v)
        nc.sync.dma_start(wq_all[:, :, 0:D // 2], wqT_v[:, :, 0:D // 2])
        cpm_sb = small.tile([P, 2 * NJ + NF], F32, tag="cpm")
        nc.sync.dma_start(cpm_sb[:], cpm)
        bq_sb, bk_sb = cpm_sb[:, 0:NJ], cpm_sb[:, NJ:2 * NJ]
        b1_sb = cpm_sb[:, 2 * NJ:2 * NJ + NF]
        nc.sync.dma_start(wq_all[:, :, ds(D // 2, D // 2)],
                          wqT_v[:, :, ds(D // 2, D // 2)])
        nc.sync.dma_start(wk_all[:], wkT_v)
        for xc in range(4):
            xchunk(xc)
        crow_sb = small.tile([1, 2 * D + 2], F32, tag="crow")
        nc.sync.dma_start(crow_sb[:], crow)
        nc.sync.dma_start(wv_all[:], wvT_v)
        wo_sb = small.tile([P, NJ, D], FP8, tag="wo")
        nc.sync.dma_start(wo_sb[:], woT_v)

        # ---------------- constants / broadcast rows ----------------
        ident = small.tile([P, P], F32, tag="ident")
        make_identity(nc, ident)
        ident_bf = small.tile([P, P], BF16, tag="ident_bf")
        nc.vector.tensor_copy(ident_bf[:], ident[:])
        ones_bf = small.tile([1, P], BF16, tag="ones_bf")
        nc.vector.memset(ones_bf[:], 1.0)
        b2row_bf = small.tile([1, D], BF16, tag="b2row_bf")
        nc.vector.tensor_copy(b2row_bf[:], crow_sb[:, ds(D, D)])

        bv_bc = small.tile([P, D], F32, tag="bv_bc")
        nc.gpsimd.partition_broadcast(bv_bc[:], crow_sb[:, 0:D])
        ag_bc = small.tile([P, 2], F32, tag="ag_bc")
        nc.gpsimd.partition_broadcast(ag_bc[:], crow_sb[:, ds(2 * D, 2)])
        alpha_bc = ag_bc[:, 0:1]
        gamma_bc = ag_bc[:, 1:2]

        eps_bc = small.tile([P, 1], F32, tag="eps_bc")
        nc.vector.memset(eps_bc[:], EPS)

        # ------------- phase A helpers (Q, K, V; fp8 DoubleRow) -------
        # Attention score/c2 psum pools open first so score emission can be
        # interleaved into the projection stream (PSUM stack: d,c2,a then
        # a closes and o,ef open -> never more than 8 banks).
        psum_d_cm = tc.tile_pool(name="psum_d", bufs=2, space="PSUM")
        psum_d = psum_d_cm.__enter__()
        psum_c2_cm = tc.tile_pool(name="psum_c2", bufs=1, space="PSUM")
        psum_c2 = psum_c2_cm.__enter__()
        psum_a_cm = tc.tile_pool(name="psum_a", bufs=3, space="PSUM")
        psum_a = psum_a_cm.__enter__()

        VN = 512
        NVN = D // VN
        HPV = VN // DK

        def qproj():
            for jt in range(NJ):
                ps = psum_a.tile([P, QN], F32, tag="ps")
                for kp in range(NJ // 2):
                    nc.tensor.matmul(
                        ps[:], wq_all[:, 2 * kp:2 * kp + 2, ts(jt, P)],
                        xTq_sb[:, 2 * kp:2 * kp + 2, :],
                        start=(kp == 0), stop=(kp == NJ // 2 - 1),
                        perf_mode=DR)
                nc.vector.tensor_scalar_add(
                    Q_sb[:, jt, :], ps[:], bq_sb[:, jt:jt + 1])

        def kproj(jt, nts=None):
            for nt in (range(NTN) if nts is None else nts):
                ps = psum_a.tile([P, TN], F32, tag="ps")
                for kp in range(NJ // 2):
                    nc.tensor.matmul(
                        ps[:], wk_all[:, 2 * kp:2 * kp + 2, ts(jt, P)],
                        xt_all[:, 2 * kp:2 * kp + 2, ds(nt * TN, TN)],
                        start=(kp == 0), stop=(kp == NJ // 2 - 1),
                        perf_mode=DR)
                nc.vector.tensor_scalar_add(
                    K_sb[:, jt, ds(nt * TN, TN)], ps[:], bk_sb[:, jt:jt + 1])

        def vproj(nv, tts):
            for tt in tts:
                ps = psum_a.tile([P, VN], F32, tag="ps")
                for kp in range(NJ // 2):
                    nc.tensor.matmul(
                        ps[:], xt_all[:, 2 * kp:2 * kp + 2, ts(tt, P)],
                        wv_all[:, 2 * kp:2 * kp + 2, ds(nv * VN, VN)],
                        start=(kp == 0), stop=(kp == NJ // 2 - 1),
                        perf_mode=DR)
                nc.vector.tensor_tensor(
                    V_sb[:, tt, ds(nv * HPV, HPV), 0:DK],
                    ps[:].rearrange("p (h d) -> p h d", d=DK),
                    bv_bc[:, ds(nv * VN, VN)].rearrange(
                        "p (h d) -> p h d", d=DK),
                    ALU.add)

        nc.sync.dma_start(xq_sb[:], xq_v)

        # ------- phases D/E/F: query-half pipelined attention + FFN -------
        # Attention runs per query half (HG=1, score chunks of 4 token
        # tiles in a 2-bank PSUM, exp -> fp8, attn.V as DoubleRow pairs).
        # While half 1's exp-bound attention occupies ACT, the PE stream is
        # filled with half 0's FFN1 strips.  O-projection accumulates all 4
        # fp8-DR k-pairs in PSUM and flushes straight into the residual.
        # FFN2 runs at the tail against streamed w2 with all 8 PSUM banks
        # as per-(tt,no) accumulators seeded with out1.
        QH2 = TQ // 2          # 256 queries per half
        CK = 4                 # score chunk: 4 token tiles per psum_d tile
        NCK = NT // CK

        exmap = {}

        def att_scores(qh, h, cks=None):
            hp = (h % HPJ) * DK
            hj = h // HPJ
            qsl = ds(qh * QH2, QH2)
            for ck in (range(NCK) if cks is None else cks):
                ps = psum_d.tile([P, CK, QH2], F32, tag="ps2")
                for i in range(CK):
                    nc.tensor.matmul(
                        ps[:, i], K_sb[ds(hp, DK), hj, ts(ck * CK + i, P)],
                        Q_sb[ds(hp, DK), hj, qsl],
                        start=True, stop=True)
                ex = expool.tile([P, CK, QH2], FP8, tag="ex",
                                 bufs=8, name=f"ex_{qh}_{h}_{ck}")
                nc.scalar.activation(ex[:], ps[:], ACTF.Exp,
                                     scale=1.0 / math.sqrt(DK))
                exmap[(qh, h, ck)] = ex

        def att_av(qh, h):
            hp = (h % HPJ) * DK
            hj = h // HPJ
            qsl = ds(qh * QH2, QH2)
            c2 = psum_c2.tile([P, QH2], F32, tag="c2", name=f"c2_{qh}_{h}")
            for ck in range(NCK):
                ex = exmap.pop((qh, h, ck))
                for i in range(CK // 2):
                    mt0 = ck * CK + 2 * i
                    nc.tensor.matmul(
                        c2[0:DK + 1, :],
                        V_sb[:, mt0:mt0 + 2, h, :], ex[:, 2 * i:2 * i + 2],
                        start=(mt0 == 0), stop=(mt0 == NT - 2),
                        perf_mode=DR)
            recip = expool.tile([1, QH2], F32, tag="recip", bufs=2)
            nc.vector.reciprocal(recip[:], c2[DK:DK + 1, :])
            recip_bc = expool.tile([DK, QH2], F32, tag="recip_bc", bufs=2)
            nc.gpsimd.partition_broadcast(recip_bc[:], recip[:])
            nc.vector.tensor_tensor(ctx_sb[ds(hp, DK), hj, qsl],
                                    c2[0:DK, :], recip_bc[:], ALU.mult)

        def attention(qh, h):
            hp = (h % HPJ) * DK
            hj = h // HPJ
            qsl = ds(qh * QH2, QH2)
            c2 = psum_c2.tile([P, QH2], F32, tag="c2", name=f"c2_{qh}_{h}")
            exs = []
            for ck in range(NCK + 1):
                if ck < NCK:
                    ps = psum_d.tile([P, CK, QH2], F32, tag="ps2")
                    for i in range(CK):
                        nc.tensor.matmul(
                            ps[:, i], K_sb[ds(hp, DK), hj, ts(ck * CK + i, P)],
                            Q_sb[ds(hp, DK), hj, qsl],
                            start=True, stop=True)
                    ex = expool.tile([P, CK, QH2], FP8, tag="ex",
                                     bufs=8, name=f"ex_{qh}_{h}_{ck}")
                    nc.scalar.activation(ex[:], ps[:], ACTF.Exp,
                                         scale=1.0 / math.sqrt(DK))
                    exs.append(ex)
                if ck >= 1:
                    ex = exs[ck - 1]
                    for i in range(CK // 2):
                        mt0 = (ck - 1) * CK + 2 * i
                        nc.tensor.matmul(
                            c2[0:DK + 1, :],
                            V_sb[:, mt0:mt0 + 2, h, :], ex[:, 2 * i:2 * i + 2],
                            start=(mt0 == 0), stop=(mt0 == NT - 2),
                            perf_mode=DR)
            recip = expool.tile([1, QH2], F32, tag="recip", bufs=2)
            nc.vector.reciprocal(recip[:], c2[DK:DK + 1, :])
            recip_bc = expool.tile([DK, QH2], F32, tag="recip_bc", bufs=2)
            nc.gpsimd.partition_broadcast(recip_bc[:], recip[:])
            nc.vector.tensor_tensor(ctx_sb[ds(hp, DK), hj, qsl],
                                    c2[0:DK, :], recip_bc[:], ALU.mult)

        def oproj_ln(qh):
            # O-projection + residual + LN1 + transpose for one query half
            for t2 in range(2):
                tt = qh * 2 + t2
                for no in range(NON):
                    pso = psum_o.tile([P, ON], F32, tag="pso",
                                      name=f"pso_{qh}_{t2}_{no}")
                    for kp in range(NJ // 2):
                        nc.tensor.matmul(
                            pso[:], ctx_sb[:, 2 * kp:2 * kp + 2, ts(tt, P)],
                            wo_sb[:, 2 * kp:2 * kp + 2, ds(no * ON, ON)],
                            start=(kp == 0), stop=(kp == NJ // 2 - 1),
                            perf_mode=DR)
                    sl = ds(no * ON, ON)
                    nc.vector.tensor_tensor(out1_sb[:, tt, sl], pso[:],
                                            xq_sb[:, tt, sl], ALU.add)
                o1 = out1_sb[:, tt, :]
                _layer_norm(nc, ev, o1, o1, D, alpha_bc, gamma_bc, eps_bc,
                            use_sqrt=(qh == 1))
                for jt in range(NJ):
                    pst = psum_ef.tile([P, QH2], F32, tag="ps1")
                    nc.tensor.transpose(
                        pst[:, 0:P], out1_sb[:, tt, ts(jt, P)], ident[:])
                    nc.vector.tensor_copy(out1T_sb[:, jt, ts(tt, P)],
                                          pst[:, 0:P])

        def ffn1_strip(qh, mp):
            qsl = ds(qh * QH2, QH2)
            w1_col = fstream.tile([P, NJ, 2 * P], BF16, tag="w1_col",
                                  name=f"w1c_{qh}_{mp}")
            nc.sync.dma_start(w1_col[:], w1T_v[:, :, ds(mp * 2 * P, 2 * P)])
            for mi in range(2):
                mt = mp * 2 + mi
                ps = psum_ef.tile([P, QH2], F32, tag="ps1")
                for kt in range(NJ):
                    nc.tensor.matmul(
                        ps[:], w1_col[:, kt, ts(mi, P)],
                        out1T_sb[:, kt, qsl],
                        start=(kt == 0), stop=(kt == NJ - 1))
                nc.vector.tensor_scalar(hid_sb[:, mt, qsl], ps[:],
                                        b1_sb[:, mt:mt + 1], 0.0,
                                        ALU.add, op1=ALU.max)

        # ---- phase A interleaved with half 0's attention ----
        # scores for head 0 are emitted right after K's first feature tile
        # so exp (the critical ACT stream) starts as early as possible; the
        # remaining K/V projection work fills the PE while ACT exps.
        nc.vector.memset(V_sb[:, :, :, DK:DK + 1], 1.0)
        qproj()
        # K feature tile 0 per token-chunk, each feeding head 0's matching
        # score chunk -> the exp stream (the ACT critical path) starts as
        # soon as the first xT chunk lands
        for ck in range(NCK):
            kproj(0, [ck])
            att_scores(0, 0, [ck])
        kproj(1)
        att_scores(0, 1)
        vproj(0, range(NT))              # heads 0-7's V (wv DMA-gated)
        att_av(0, 0)
        att_scores(0, 2)
        att_av(0, 1)
        for jt in range(2, NJ):
            kproj(jt)
            if jt < NJ - 1:
                att_scores(0, jt + 1)
                att_av(0, jt)
        att_av(0, NJ - 1)
        vproj(1, range(NT))              # heads 8-15's V
        psum_a_cm.__exit__(None, None, None)
        psum_o_cm = tc.tile_pool(name="psum_o", bufs=1, space="PSUM")
        psum_o = psum_o_cm.__enter__()
        psum_ef_cm = tc.tile_pool(name="psum_ef", bufs=2, space="PSUM")
        psum_ef = psum_ef_cm.__enter__()
        xt_cm.__exit__(None, None, None)    # release xT + proj weights
        fpool_cm = tc.tile_pool(name="fpool", bufs=1, side="right")
        fpool = fpool_cm.__enter__()
        hid_sb = fpool.tile([P, NF, TQ], BF16, tag="hid")
        fstream_cm = tc.tile_pool(name="fstream", bufs=4, side="right")
        fstream = fstream_cm.__enter__()
        w2_blk0 = fstream.tile([P, 4, D], BF16, tag="w2_blk0", bufs=1)
        nc.gpsimd.dma_start(w2_blk0[:], w2T_v[:, 0:4, :])
        for h in range(H // 2, H):
            attention(0, h)
        oproj_ln(0)
        # ---- half 1: attention interleaved with half 0's FFN1 ----
        for h in range(H):
            attention(1, h)
            ffn1_strip(0, h)
        oproj_ln(1)
        for mp in range(NF // 2):
            ffn1_strip(1, mp)
        psum_ef_cm.__exit__(None, None, None)
        psum_o_cm.__exit__(None, None, None)
        psum_c2_cm.__exit__(None, None, None)
        psum_d_cm.__exit__(None, None, None)
        expool_cm.__exit__(None, None, None)

        v_cm.__exit__(None, None, None)     # release V
        kq_cm.__exit__(None, None, None)    # release K, Q

        # ---------------- FFN2 tail: streamed w2, 8 psum accumulators ------
        KTG = 4
        NKTG = NF // KTG
        f2s_cm = tc.tile_pool(name="f2stream", bufs=2, side="right")
        f2s = f2s_cm.__enter__()
        psum_f2_cm = tc.tile_pool(name="psum_f2", bufs=8, space="PSUM")
        psum_f2 = psum_f2_cm.__enter__()
        psf = {}
        for tt in range(NTQ):
            for no in range(NON):
                ps = psum_f2.tile([P, ON], F32, tag="psf",
                                  name=f"psf_{tt}_{no}")
                psf[(tt, no)] = ps
                for jj in range(ON // P):
                    jt = no * (ON // P) + jj
                    nc.tensor.matmul(
                        ps[:, ds(jj * P, P)], out1T_sb[:, jt, ts(tt, P)],
                        ident_bf[:], start=(jj == 0), stop=False,
                        skip_group_check=True)
                nc.tensor.matmul(
                    ps[:], ones_bf[:], b2row_bf[:, ds(no * ON, ON)],
                    start=False, stop=False, skip_group_check=True)
        for ktg in range(NKTG - 1):
            if ktg == 0:
                w2_blk = w2_blk0
            else:
                w2_blk = f2s.tile([P, KTG, D], BF16, tag="w2_blk")
                nc.gpsimd.dma_start(w2_blk[:], w2T_v[:, ds(ktg * KTG, KTG), :])
            for tt in range(NTQ):
                for no in range(NON):
                    ps = psf[(tt, no)]
                    for kk in range(KTG):
                        kt = ktg * KTG + kk
                        nc.tensor.matmul(
                            ps[:], hid_sb[:, kt, ts(tt, P)],
                            w2_blk[:, kk, ds(no * ON, ON)],
                            start=False,
                            stop=False,
                            skip_group_check=True)
        # last block: per-tile finish so each LN2 chain overlaps the next
        # tile's matmuls (short drain)
        w2_blk = f2s.tile([P, KTG, D], BF16, tag="w2_blk")
        nc.gpsimd.dma_start(w2_blk[:], w2T_v[:, ds(NF - KTG, KTG), :])
        for tt in range(NTQ):
            res2 = ev.tile([P, D], F32, tag="res2")
            st = ev.tile([P, 16], F32, tag="f2stat")
            sqg = ev.tile([P, ON], BF16, tag="f2sq", bufs=1)
            for no in range(NON):
                ps = psf[(tt, no)]
                for kk in range(KTG):
                    kt = NF - KTG + kk
                    nc.tensor.matmul(
                        ps[:], hid_sb[:, kt, ts(tt, P)],
                        w2_blk[:, kk, ds(no * ON, ON)],
                        start=False, stop=(kt == NF - 1),
                        skip_group_check=True)
                # stats straight off the psum accumulator (b2 was seeded)
                nc.vector.reduce_sum(st[:, no:no + 1], ps[:], axis=AX.X)
                nc.scalar.activation(sqg[:], ps[:], ACTF.Square,
                                     accum_out=st[:, 2 + no:3 + no])
            nc.vector.tensor_tensor(st[:, 4:5], st[:, 0:1], st[:, 1:2],
                                    ALU.add)                       # sum
            nc.vector.tensor_tensor(st[:, 5:6], st[:, 2:3], st[:, 3:4],
                                    ALU.add)                       # sumsq
            nc.vector.tensor_scalar(st[:, 6:7], st[:, 4:5], st[:, 4:5],
                                    1.0 / (D * D), ALU.mult, op1=ALU.mult)
            nc.vector.tensor_scalar(st[:, 7:8], st[:, 5:6], 1.0 / D, EPS,
                                    ALU.mult, op1=ALU.add)
            nc.vector.tensor_tensor(st[:, 8:9], st[:, 7:8], st[:, 6:7],
                                    ALU.subtract)                  # var+eps
            nc.scalar.activation(st[:, 9:10], st[:, 8:9], ACTF.Sqrt)
            nc.vector.reciprocal(st[:, 10:11], st[:, 9:10])
            nc.vector.tensor_scalar(st[:, 11:12], st[:, 10:11], alpha_bc,
                                    None, ALU.mult)                # s
            nc.vector.tensor_scalar(st[:, 12:13], st[:, 11:12], st[:, 4:5],
                                    1.0 / D, ALU.mult, op1=ALU.mult)
            nc.vector.tensor_scalar(st[:, 13:14], st[:, 12:13], -1.0,
                                    gamma_bc, ALU.mult, op1=ALU.add)  # t
            for no in range(NON):
                sl = ds(no * ON, ON)
                nc.vector.tensor_scalar(res2[:, sl], psf[(tt, no)][:],
                                        st[:, 11:12], st[:, 13:14],
                                        ALU.mult, op1=ALU.add)
                nc.sync.dma_start(out_v[:, tt, sl], res2[:, sl])
        psum_f2_cm.__exit__(None, None, None)
        f2s_cm.__exit__(None, None, None)
        fstream_cm.__exit__(None, None, None)
        fpool_cm.__exit__(None, None, None)
        ctx_cm.__exit__(None, None, None)

        ev_cm.__exit__(None, None, None)
        ef_cm.__exit__(None, None, None)

    return nc


def _layer_norm(nc, pool, out_ap, x_ap, D, alpha_bc, gamma_bc, eps_bc,
                use_sqrt=False, final_pool=False):
    """out = alpha * (x - mean) / sqrt(var + EPS) + gamma, stats over free dim.

    One-pass stats: sum on DVE and sum-of-squares on ACT run in parallel;
    var = sumsq/D - mean^2; out = x*(alpha*rstd) + (gamma - mean*alpha*rstd)
    is a single ACT pass over x with per-partition scale/bias.
    """
    stat = pool.tile([P, 8], F32, tag="ln_stat")
    sq = pool.tile([P, D], BF16, tag="ln_sq", bufs=1)
    nc.vector.reduce_sum(stat[:, 0:1], x_ap, axis=AX.X)
    nc.scalar.activation(sq[:], x_ap, ACTF.Square, accum_out=stat[:, 2:3])
    # m2 = (sum/D)^2 ; ve = sumsq/D + eps ; var' = ve - m2
    nc.vector.tensor_scalar(stat[:, 3:4], stat[:, 0:1], stat[:, 0:1],
                            1.0 / (D * D), ALU.mult, op1=ALU.mult)
    nc.vector.tensor_scalar(stat[:, 4:5], stat[:, 2:3], 1.0 / D, EPS,
                            ALU.mult, op1=ALU.add)
    nc.vector.tensor_tensor(stat[:, 5:6], stat[:, 4:5], stat[:, 3:4],
                            ALU.subtract)
    rstd = pool.tile([P, 8], F32, tag="ln_rstd")
    v = stat[:, 5:6]
    z = rstd[:, 1:2]
    if use_sqrt:
        # tail path: exp is over, one table switch to sqrt is cheap and
        # much shorter than the Newton chain
        nc.scalar.activation(rstd[:, 0:1], v, ACTF.Sqrt)
        nc.vector.reciprocal(z, rstd[:, 0:1])
    else:
        # mid-kernel path: 1/sqrt(var) via reciprocal seed + 3 Newton
        # iterations, entirely off the ACT table path (no table swaps
        # around the softmax exp stream)
        nc.vector.reciprocal(rstd[:, 0:1], v)
        nc.vector.tensor_scalar(z, rstd[:, 0:1], 0.5, 0.5, ALU.mult,
                                op1=ALU.add)
        for it in range(3):
            u = rstd[:, 2 + it:3 + it]
            nc.vector.tensor_tensor(u, z, z, ALU.mult)        # z^2
            nc.vector.tensor_scalar(u, u, v, None, ALU.mult)  # v*z^2
            nc.vector.tensor_scalar(u, u, -0.5, 1.5, ALU.mult, op1=ALU.add)
            nc.vector.tensor_scalar(z, z, u, None, ALU.mult)
    s = rstd[:, 5:6]
    nc.vector.tensor_scalar(s, z, alpha_bc, None, ALU.mult)
    # t = gamma - (sum/D)*s
    nc.vector.tensor_scalar(rstd[:, 6:7], s, stat[:, 0:1],
                            1.0 / D, ALU.mult, op1=ALU.mult)
    nc.vector.tensor_scalar(rstd[:, 7:8], rstd[:, 6:7], -1.0, gamma_bc,
                            ALU.mult, op1=ALU.add)
    nc.vector.tensor_scalar(out_ap, x_ap, s, rstd[:, 7:8],
                            ALU.mult, op1=ALU.add)

_B, _S, _D, _H, _DK, _DFF = 2, 2048, 1024, 16, 64, 4096
_NCORES = 8
_TQ = (_B * _S) // _NCORES    # 512 query tokens per core

_cache = {}


def _get_program():
    if "nc" not in _cache:
        from concourse import bacc
        nc = bacc.Bacc("TRN2", target_bir_lowering=False, debug=False,
                       num_devices=_NCORES)
        build(nc, S=_S, D=_D, H=_H, DK=_DK, DFF=_DFF, TQ=_TQ)
        nc.compile()
        _cache["nc"] = nc
    return _cache["nc"]


def _core_inputs(inp):
    """Host-side prep: per-core input dicts (transposes + dtype casts only)."""
    import ml_dtypes
    bf = ml_dtypes.bfloat16
    f8 = ml_dtypes.float8_e4m3

    def t_cast(a, dt):
        return np.ascontiguousarray(np.asarray(a).T).astype(dt)

    bo = np.asarray(inp["bo"])
    f32 = np.float32
    pm = lambda a: np.asarray(a, f32).reshape(-1, 128).T   # partition-major
    cpm = np.ascontiguousarray(np.concatenate(
        [pm(inp["bq"]), pm(inp["bk"]), pm(inp["b1"])], axis=1))
    crow = np.concatenate(
        [np.asarray(inp["bv"], f32), np.asarray(inp["b2"], f32),
         np.asarray(inp["alpha"], f32),
         np.asarray(inp["gamma"], f32)])[None, :]
    w = {
        "wqT": t_cast(inp["wq"], f8), "wkT": t_cast(inp["wk"], f8),
        "wvT": t_cast(inp["wv"], f8), "woT": t_cast(inp["wo"], f8),
        "w1T": t_cast(inp["w1"], bf), "w2T": t_cast(inp["w2"], bf),
        "cpm": cpm, "crow": np.ascontiguousarray(crow),
    }
    x = np.asarray(inp["x"])
    per_batch = _NCORES // _B
    maps = []
    for c in range(_NCORES):
        b, q0 = c // per_batch, (c % per_batch) * _TQ
        xb = x[b]
        m = dict(w)
        m["xT"] = np.ascontiguousarray(xb.T).astype(f8)
        m["xTq"] = np.ascontiguousarray(xb[q0:q0 + _TQ].T).astype(f8)
        # fold the O-projection bias into the residual stream input
        m["xq"] = np.ascontiguousarray(xb[q0:q0 + _TQ]) + bo[None, :]
        maps.append(m)
    return maps


def kernel(**inputs) -> np.ndarray:
    from concourse.bass_utils import run_bass_kernel_spmd
    nc = _get_program()
    in_maps = _core_inputs(inputs)
    res = run_bass_kernel_spmd(nc, in_maps, core_ids=list(range(_NCORES)))
    out = np.empty((_B, _S, _D), dtype=np.float32)
    per_batch = _NCORES // _B
    for c, rm in enumerate(res.results):
        b, q0 = c // per_batch, (c % per_batch) * _TQ
        out[b, q0:q0 + _TQ] = rm["out"]
    return out
